# revision 2
# baseline (speedup 1.0000x reference)
import sys
sys.path.insert(0, '/opt/trn_rl_repo')
from contextlib import ExitStack

import numpy as np
import ml_dtypes
BF_NP = ml_dtypes.bfloat16

import concourse.bass as bass
import concourse.bacc as bacc
import concourse.mybir as mybir
from concourse.bass import broadcast_tensor_aps
from concourse.bass_utils import run_bass_kernel_spmd
from concourse.tile import TileContext

F32 = mybir.dt.float32
BF16 = mybir.dt.bfloat16
AF = mybir.ActivationFunctionType
OP = mybir.AluOpType

C, L, DI, N, DTR = 64, 128, 128, 16, 4
SB = 2                  # sequences per block
TOK = SB * L            # tokens per block = 256
LAT = N * TOK           # lattice free size per block = 4096
NBLK = L // SB          # 64 blocks per pass
HW = L * L              # 16384
HHW = HW // 2           # packed free size = 8192
GN_EPS = 1e-5
NCORES = 8
CHK = 512               # psum broadcast chunk width
NCHUNK = LAT // CHK     # 4

_CACHE = {}
PROFILE = False


def _build(a_row, a_col):
    # a_row/a_col: python float lists (len N) = A diag values per state
    nc = bacc.Bacc()
    x = nc.dram_tensor("x", (C, HW), BF16, kind="ExternalInput")
    out = nc.dram_tensor("out", (C, HW), F32, kind="ExternalOutput")
    pr = {}
    for ax in ("r", "c"):
        pr[ax] = dict(
            # conv-fused in_proj stationaries: wx scaled by each conv tap
            wxk=[nc.dram_tensor(f"{ax}_wxk{j}", (C, DI), BF16,
                                kind="ExternalInput") for j in range(4)],
            wz=nc.dram_tensor(f"{ax}_wz", (C, DI), BF16, kind="ExternalInput"),
            xp=nc.dram_tensor(f"{ax}_xp", (DI, DTR + 2 * N), BF16,
                              kind="ExternalInput"),
            dtw=nc.dram_tensor(f"{ax}_dtw", (DTR, DI), BF16,
                               kind="ExternalInput"),
            dtb=nc.dram_tensor(f"{ax}_dtb", (DI, 1), F32,
                               kind="ExternalInput"),
            cvb=nc.dram_tensor(f"{ax}_cvb", (DI, 1), F32,
                               kind="ExternalInput"),
            Dp=nc.dram_tensor(f"{ax}_Dp", (DI, 1), F32, kind="ExternalInput"),
            ow=nc.dram_tensor(f"{ax}_ow", (DI, C), BF16, kind="ExternalInput"),
        )
    selg = nc.dram_tensor("selg", (C * 2, 4), F32, kind="ExternalInput")
    selc = nc.dram_tensor("selc", (4, C * 2), F32, kind="ExternalInput")
    gnw = nc.dram_tensor("gnw", (C * 2, 1), F32, kind="ExternalInput")
    gnb = nc.dram_tensor("gnb", (C * 2, 1), F32, kind="ExternalInput")

    A_of = {"r": a_row, "c": a_col}

    with TileContext(nc) as tc:
        with ExitStack() as ctx:
            cpool = ctx.enter_context(tc.tile_pool(name="consts", bufs=1))
            spool = ctx.enter_context(tc.tile_pool(name="small", bufs=3))
            lpool = ctx.enter_context(tc.tile_pool(name="lat", bufs=2))
            fpool = ctx.enter_context(tc.tile_pool(name="bcf", bufs=2))
            xpool = ctx.enter_context(tc.tile_pool(name="xrec", bufs=1))
            mpool = ctx.enter_context(tc.tile_pool(name="pmm", bufs=2,
                                                   space="PSUM"))
            bpool = ctx.enter_context(tc.tile_pool(name="pbc", bufs=4,
                                                   space="PSUM"))
            qpool = ctx.enter_context(tc.tile_pool(name="pxd", bufs=1,
                                                   space="PSUM"))
            opool = ctx.enter_context(tc.tile_pool(name="po", bufs=1,
                                                   space="PSUM"))

            cs = {}
            for ax in ("r", "c"):
                p = pr[ax]
                cs[ax] = dict(
                    wxk=[cpool.tile_from(p["wxk"][j][:], name=f"{ax}wxk{j}")
                         for j in range(4)],
                    wz=cpool.tile_from(p["wz"][:], name=f"{ax}wz"),
                    xp=cpool.tile_from(p["xp"][:], name=f"{ax}xp"),
                    dtw=cpool.tile_from(p["dtw"][:], name=f"{ax}dtw"),
                    dtb=cpool.tile_from(p["dtb"][:], name=f"{ax}dtb"),
                    cvb=cpool.tile_from(p["cvb"][:], name=f"{ax}cvb"),
                    Dp=cpool.tile_from(p["Dp"][:], name=f"{ax}Dp"),
                    ow=cpool.tile_from(p["ow"][:], name=f"{ax}ow"),
                )
            selg_s = cpool.tile_from(selg[:], name="selg")
            selc_s = cpool.tile_from(selc[:], name="selc")
            gnw_s = cpool.tile_from(gnw[:], name="gnw")
            gnb_s = cpool.tile_from(gnb[:], name="gnb")
            ones_b = cpool.tile([1, DI], BF16, name="ones")
            onef = cpool.tile([1, DI], F32, name="onef")
            nc.vector.memzero(onef[:])
            nc.vector.tensor_scalar_add(onef[:], onef[:], 1.0)
            nc.vector.tensor_copy(ones_b[:], onef[:])

            # packed accumulator: partition p = c + 64*j, free = hw - j*HHW
            xrec = xpool.tile([2 * C, HHW], F32)
            nc.vector.memzero(xrec[:])

            xcol = x[:].rearrange("c (h w) -> c w h", w=L)

            for ax, rev in (("r", 0), ("r", 1), ("c", 0), ("c", 1)):
                kp = cs[ax]
                Avals = A_of[ax]
                for i in range(NBLK):
                    tok = spool.tile([C, TOK], BF16, tag="tok", bufs=6)
                    if ax == "r":
                        nc.sync.dma_start(tok[:], x[:, i * TOK:(i + 1) * TOK])
                    else:
                        tok_src = xcol[:, SB * i:SB * (i + 1), :]
                        for s in range(SB):
                            nc.sync.dma_start(tok[:, s * L:(s + 1) * L],
                                              tok_src[:, s, :])

                    # in_proj + causal depthwise conv fused on PE:
                    # xc[d,(s,t)] = sum_j wxk[j][c,d] tok[c,(s,t-j)] (fwd)
                    ps_x = mpool.tile([DI, TOK], F32, tag="mm")
                    tok3 = tok[:].rearrange("c (s t) -> c s t", s=SB)
                    px3 = ps_x[:].rearrange("d (s t) -> d s t", s=SB)
                    nc.tensor.matmul(ps_x[:], kp["wxk"][0][:], tok[:],
                                     start=True, stop=False)
                    for j in (1, 2, 3):
                        if not rev:
                            o_ap, i_ap = px3[:, :, j:L], tok3[:, :, 0:L - j]
                        else:
                            o_ap, i_ap = px3[:, :, 0:L - j], tok3[:, :, j:L]
                        nc.tensor.matmul(o_ap, kp["wxk"][j][:], i_ap,
                                         start=False, stop=(j == 3),
                                         skip_group_check=True)
                    xt = spool.tile([DI, TOK], BF16, tag="xt", bufs=4)
                    nc.scalar.activation(xt[:], ps_x[:], AF.Silu,
                                         bias=kp["cvb"][:])

                    ps_z = mpool.tile([DI, TOK], F32, tag="mm")
                    nc.tensor.matmul(ps_z[:], kp["wz"][:], tok[:],
                                     start=True, stop=True)
                    zs = spool.tile([DI, TOK], BF16, tag="zs", bufs=4)
                    nc.scalar.activation(zs[:], ps_z[:], AF.Silu)

                    ps_xd = qpool.tile([DTR + 2 * N, TOK], F32, tag="psxd")
                    nc.tensor.matmul(ps_xd[:], kp["xp"][:], xt[:],
                                     start=True, stop=True)
                    dt_sb = spool.tile([DTR, TOK], BF16, tag="dt", bufs=2)
                    nc.vector.tensor_copy(dt_sb[:], ps_xd[32:32 + DTR, :])
                    bc16 = spool.tile([2 * N, TOK], BF16, tag="bc16", bufs=2)
                    nc.vector.tensor_copy(bc16[:], ps_xd[0:2 * N, :])
                    bcf = fpool.tile([1, 2 * N * TOK], BF16, tag="bcf")
                    nc.scalar.dma_start(bcf[:], bc16[:])

                    ps_d = mpool.tile([DI, TOK], F32, tag="mm")
                    nc.tensor.matmul(ps_d[:], kp["dtw"][:], dt_sb[:],
                                     start=True, stop=True)
                    esb = spool.tile([DI, TOK], F32, tag="esb", bufs=2)
                    nc.scalar.activation(esb[:], ps_d[:], AF.Exp,
                                         bias=kp["dtb"][:])
                    delta = spool.tile([DI, TOK], F32, tag="delta", bufs=2)
                    nc.scalar.activation(delta[:], esb[:], AF.Ln, bias=1.0)
                    du = spool.tile([DI, TOK], F32, tag="du", bufs=2)
                    nc.gpsimd.tensor_tensor(du[:], delta[:], xt[:], OP.mult)

                    dA = lpool.tile([DI, LAT], BF16, tag="dA")
                    for n in range(N):
                        nc.scalar.activation(dA[:, n * TOK:(n + 1) * TOK],
                                             delta[:], AF.Exp,
                                             scale=float(Avals[n]))
                    dA3 = dA[:].rearrange("d (q t) -> d q t", t=L)
                    zc = dA3[:, :, 0:1] if not rev else dA3[:, :, L - 1:L]
                    nc.vector.tensor_scalar_mul(zc, zc, 0.0)

                    dBu = lpool.tile([DI, LAT], F32, tag="dBu", bufs=1)
                    du3 = du[:].rearrange("d (o t) -> d o t", o=1)
                    npc = CHK // TOK
                    for cI in range(NCHUNK):
                        bch = bpool.tile([DI, CHK], F32, tag="bc")
                        nc.tensor.matmul(
                            bch[:], ones_b[:],
                            bcf[0:1, cI * CHK:(cI + 1) * CHK],
                            start=True, stop=True)
                        dview = dBu[:, cI * CHK:(cI + 1) * CHK].rearrange(
                            "d (n t) -> d n t", n=npc)
                        bview = bch[:].rearrange("d (n t) -> d n t", n=npc)
                        du_b, _ = broadcast_tensor_aps(du3, dview)
                        nc.vector.tensor_tensor(dview, bview, du_b, OP.mult)

                    h = lpool.tile([DI, LAT], F32, tag="h", bufs=1)
                    if not rev:
                        nc.vector.tensor_tensor_scan(
                            h[:], dA[:], dBu[:], 0.0, OP.mult, OP.add)
                    else:
                        nc.vector.tensor_tensor_scan(
                            h[:, ::-1], dA[:, ::-1], dBu[:, ::-1], 0.0,
                            OP.mult, OP.add)

                    hC = lpool.tile([DI, LAT], F32, tag="dBu", bufs=1)
                    for cI in range(NCHUNK):
                        cch = bpool.tile([DI, CHK], F32, tag="bc")
                        nc.tensor.matmul(
                            cch[:], ones_b[:],
                            bcf[0:1, LAT + cI * CHK:LAT + (cI + 1) * CHK],
                            start=True, stop=True)
                        nc.vector.tensor_tensor(hC[:, cI * CHK:(cI + 1) * CHK],
                                                h[:, cI * CHK:(cI + 1) * CHK],
                                                cch[:], OP.mult)

                    # tree-reduce over n (outer axis): y = hC[:, 0:TOK]
                    w = LAT
                    while w > TOK:
                        nc.vector.tensor_tensor(hC[:, 0:w // 2],
                                                hC[:, 0:w // 2],
                                                hC[:, w // 2:w], OP.add)
                        w //= 2


                    y2 = spool.tile([DI, TOK], F32, tag="y2", bufs=2)
                    nc.vector.scalar_tensor_tensor(
                        y2[:], xt[:], kp["Dp"][:], hC[:, 0:TOK],
                        OP.mult, OP.add)
                    y2b = spool.tile([DI, TOK], BF16, tag="y2b", bufs=2)
                    nc.gpsimd.tensor_tensor(y2b[:], y2[:], zs[:], OP.mult)

                    ps_o = opool.tile([C, TOK], F32, tag="po")
                    nc.tensor.matmul(ps_o[:], kp["ow"][:], y2b[:],
                                     start=True, stop=True)
                    if ax == "r":
                        half, off = (0, i * TOK) if i < NBLK // 2 else \
                                    (C, (i - NBLK // 2) * TOK)
                        dst = xrec[half:half + C, off:off + TOK]
                        nc.vector.tensor_tensor(dst, dst, ps_o[:], OP.add)
                    else:
                        po3 = ps_o[:].rearrange("c (s t) -> c s t", s=SB)
                        for half in (0, 1):
                            xv = xrec[half * C:(half + 1) * C, :].rearrange(
                                "c (hh w) -> c w hh", w=L)
                            dst = xv[:, SB * i:SB * (i + 1), :]
                            srcv = po3[:, :, half * (L // 2):(half + 1) * (L // 2)]
                            nc.vector.tensor_tensor(dst, dst, srcv, OP.add)

            # --- GroupNorm(4) + SiLU + residual, on packed xrec ---
            NCH = 8
            GCH = HHW // NCH
            stats = spool.tile([2 * C, 2 * NCH], F32, tag="stats")
            for j in range(NCH):
                ch = xrec[:, j * GCH:(j + 1) * GCH]
                nc.vector.tensor_reduce(stats[:, j:j + 1], ch,
                                        mybir.AxisListType.X, OP.add)
                sq = lpool.tile([2 * C, GCH], F32, tag="dBu", bufs=1)
                nc.vector.tensor_tensor(sq[:], ch, ch, OP.mult)
                nc.vector.tensor_reduce(stats[:, NCH + j:NCH + j + 1], sq[:],
                                        mybir.AxisListType.X, OP.add)
            st2 = spool.tile([2 * C, 2], F32, tag="st2")
            nc.vector.tensor_reduce(
                st2[:], stats[:].rearrange("c (a j) -> c a j", a=2),
                mybir.AxisListType.X, OP.add)
            ps_g = opool.tile([4, 2], F32, tag="po")
            nc.tensor.matmul(ps_g[:], selg_s[:], st2[:], start=True, stop=True)
            mv = spool.tile([4, 2], F32, tag="mv")
            nc.vector.tensor_scalar_mul(mv[:], ps_g[:], 1.0 / (16 * HW))
            mu = mv[:, 0:1]
            var = spool.tile([4, 1], F32, tag="var")
            nc.vector.tensor_tensor(var[:], mu, mu, OP.mult)
            nc.vector.tensor_tensor(var[:], mv[:, 1:2], var[:], OP.subtract)
            sd = spool.tile([4, 1], F32, tag="sd")
            nc.vector.tensor_scalar_add(var[:], var[:], GN_EPS)
            nc.scalar.activation(sd[:], var[:], AF.Sqrt)
            rs = spool.tile([4, 1], F32, tag="rs")
            nc.vector.reciprocal(rs[:], sd[:])
            murs = spool.tile([4, 2], F32, tag="mv")
            nc.vector.tensor_copy(murs[:, 0:1], mu)
            nc.vector.tensor_copy(murs[:, 1:2], rs[:])
            ps_c = opool.tile([2 * C, 2], F32, tag="po")
            nc.tensor.matmul(ps_c[:], selc_s[:], murs[:], start=True, stop=True)
            aa = spool.tile([2 * C, 1], F32, tag="aa")
            nc.vector.tensor_tensor(aa[:], ps_c[:, 1:2], gnw_s[:], OP.mult)
            bb = spool.tile([2 * C, 1], F32, tag="bb")
            nc.vector.tensor_tensor(bb[:], ps_c[:, 0:1], aa[:], OP.mult)
            nc.vector.tensor_tensor(bb[:], gnb_s[:], bb[:], OP.subtract)
            for j in range(NCH):
                sl = slice(j * GCH, (j + 1) * GCH)
                sil = lpool.tile([2 * C, GCH], F32, tag="sil", bufs=2)
                nc.scalar.activation(sil[:], xrec[:, sl],
                                     AF.Silu, scale=aa[:], bias=bb[:])
                xres = lpool.tile([2 * C, GCH], BF16, tag="xres", bufs=2)
                nc.sync.dma_start(xres[0:C, :], x[:, j * GCH:(j + 1) * GCH])
                nc.sync.dma_start(xres[C:2 * C, :],
                                  x[:, HHW + j * GCH:HHW + (j + 1) * GCH])
                if j % 2 == 0:
                    nc.vector.tensor_tensor(sil[:], sil[:], xres[:], OP.add)
                else:
                    nc.gpsimd.tensor_tensor(sil[:], sil[:], xres[:], OP.add)
                nc.sync.dma_start(out[:, j * GCH:(j + 1) * GCH], sil[0:C, :])
                nc.sync.dma_start(out[:, HHW + j * GCH:HHW + (j + 1) * GCH],
                                  sil[C:2 * C, :])
    nc.compile()
    return nc


def _prep(axp):
    in_w, conv_w, conv_b, xp_w, dt_w, dt_b, A_log, Dp, out_w = axp
    d = {}
    wx = np.ascontiguousarray(in_w[:DI, :].T.astype(np.float32))  # (C, DI)
    wk = conv_w[:, 0, :].astype(np.float32)                       # (DI, 4)
    # tap j uses conv weight index 3-j (causal, newest tap = idx 3)
    for j in range(4):
        d[f"wxk{j}"] = np.ascontiguousarray((wx * wk[None, :, 3 - j]).astype(BF_NP))
    d["wz"] = np.ascontiguousarray(in_w[DI:2 * DI, :].T.astype(BF_NP))
    xp_r = np.concatenate([xp_w[DTR:], xp_w[:DTR]], axis=0)  # [B,C,dt] order
    d["xp"] = np.ascontiguousarray(xp_r.T.astype(BF_NP))
    d["dtw"] = np.ascontiguousarray(dt_w.T.astype(BF_NP))
    d["dtb"] = dt_b.astype(np.float32).reshape(DI, 1)
    d["cvb"] = conv_b.astype(np.float32).reshape(DI, 1)
    d["Dp"] = Dp.astype(np.float32).reshape(DI, 1)
    d["ow"] = np.ascontiguousarray((0.25 * out_w).T.astype(BF_NP))
    A = (-np.exp(A_log)).astype(np.float64)
    assert np.allclose(A, A[0:1, :], atol=1e-6), "A varies per channel"
    d["_Avals"] = [float(v) for v in A[0]]
    return d


def kernel(**inputs):
    x = np.asarray(inputs["x"], np.float32)
    b = x.shape[0]
    names = ("in_w", "conv_w", "conv_b", "xp_w", "dt_w", "dt_b", "A_log",
             "D", "out_w")
    rp = _prep([np.asarray(inputs["row_" + n]) for n in names])
    cp = _prep([np.asarray(inputs["col_" + n]) for n in names])

    key = (tuple(rp["_Avals"]), tuple(cp["_Avals"]))
    if _CACHE.get("key") != key:
        _CACHE["nc"] = _build(rp["_Avals"], cp["_Avals"])
        _CACHE["key"] = key
    nc = _CACHE["nc"]

    base = {}
    for k, v in rp.items():
        if k.startswith("_"):
            continue
        base["r_" + k] = v
    for k, v in cp.items():
        if k.startswith("_"):
            continue
        base["c_" + k] = v
    selg = np.zeros((2 * C, 4), np.float32)
    for p in range(2 * C):
        selg[p, (p % C) // 16] = 1.0
    base["selg"] = selg
    base["selc"] = np.ascontiguousarray(selg.T)
    gw = np.asarray(inputs["gn_w"], np.float32)
    gb = np.asarray(inputs["gn_b"], np.float32)
    base["gnw"] = np.concatenate([gw, gw]).reshape(2 * C, 1)
    base["gnb"] = np.concatenate([gb, gb]).reshape(2 * C, 1)

    in_maps = []
    for i in range(NCORES):
        m = dict(base)
        m["x"] = np.ascontiguousarray(x[i % b].reshape(C, HW).astype(BF_NP))
        in_maps.append(m)
    res = run_bass_kernel_spmd(nc, in_maps, list(range(NCORES)),
                               trace=PROFILE)
    if PROFILE and res.exec_time_ns is not None:
        print(f"HW exec time: {res.exec_time_ns} ns")
        _CACHE["exec_time_ns"] = res.exec_time_ns
        _CACHE["trace"] = res.instructions_and_trace
    outs = [np.asarray(res.results[i]["out"], np.float32).reshape(C, L, L)
            for i in range(b)]
    return np.stack(outs, 0).astype(np.float32)
